# revision 1
# baseline (speedup 1.0000x reference)
"""Trainium2 Bass kernel for nn_DiffKS (differentiable Karplus-Strong).

Structure of the computation:
  1. Frame-rate params (250 frames) are upsampled to sample rate with natural
     cubic splines; per-sample 3-tap IIR coefficients (g1,g2,g3) and integer
     delays z in [89, 317] are derived.  This is tiny O(N) host work, done in
     float64 numpy.
  2. The hard part is the strictly sequential 131072-step recursion
         y[t] = x[t] + g1*y[t-z-1] + g2*y[t-z-2] + g3*y[t-z-3].
     Because every tap lag is >= 90, outputs are computed in chunks of W=88
     samples: all samples of a chunk depend only on earlier chunks.  The
     signal is stored column-major [88 x n_chunks] in SBUF and each chunk is
     produced by 1-3 fp32 tensor-engine matmuls
         y_col[m] = sum_c A_c @ y_col[m-c]   (c in 1..4)
     against host-precomputed dense banded weight blocks (lhsT layout
     [89, 88]; the extra row carries the excitation x against a constant-ones
     row of the rhs, so PSUM accumulates x for free).  PSUM is then evicted
     to the SBUF y-column by the scalar engine, and the tensor engine
     continues with the next chunk.  Weight blocks stream from HBM in
     double-buffered group DMAs.
"""

import ml_dtypes
import numpy as np

import concourse.bass as bass
import concourse.mybir as mybir
import concourse.tile as tile
from concourse import bacc
from concourse.bass_utils import run_bass_kernel_spmd

W = 88          # chunk width (<= min tap lag, which is 90 for these inputs)
LEAD = 4        # zero history columns before chunk 0 (max lag 320 < 4*88)
KROW = W + 1    # weight block rows: W history samples + 1 excitation row
BG = 64         # bf16 weight slots per DMA group (2 slots per logical block)
F32 = mybir.dt.float32
BF16 = mybir.dt.bfloat16
BF16NP = ml_dtypes.bfloat16
N_CORES = 8


# ----------------------------------------------------------------- host math
def _host_preprocess(delay_frames, raw_coeff, excitation, n_samples):
    dt = np.float64
    F = delay_frames.shape[0]
    sig = 1.0 / (1.0 + np.exp(-raw_coeff.astype(dt)))
    coeff = sig / sig.sum(-1, keepdims=True)
    t_in = np.linspace(0.0, 1.0, F).astype(dt)
    t_out = np.linspace(0.0, 1.0, n_samples).astype(dt)
    x = np.concatenate([delay_frames.astype(dt)[:, None], coeff], axis=1)
    h = t_in[1:] - t_in[:-1]
    hinv = 1.0 / h
    dx3 = 3.0 * (x[1:] - x[:-1])
    rhs_part = dx3 * (hinv * hinv)[:, None]
    diag = np.zeros(F, dt)
    diag[:-1] += hinv
    diag[1:] += hinv
    diag *= 2.0
    rhs = np.zeros_like(x)
    rhs[:-1] += rhs_part
    rhs[1:] += rhs_part
    M = np.diag(diag) + np.diag(hinv, 1) + np.diag(hinv, -1)
    k = np.linalg.solve(M, rhs)
    hc = hinv[:, None]
    a = x[:-1]
    b = k[:-1]
    two_c = (2.0 * dx3 * hc - 4.0 * k[:-1] - 2.0 * k[1:]) * hc
    three_d = (-2.0 * dx3 * hc + 3.0 * (k[:-1] + k[1:])) * hc * hc
    idx = np.clip(np.searchsorted(t_in, t_out, side="left") - 1, 0, F - 2)
    f = (t_out - t_in[idx])[:, None]
    inner = b[idx] + (0.5 * two_c[idx] + three_d[idx] * (f / 3.0)) * f
    vals = a[idx] + inner * f
    delay = vals[:, 0]
    b1 = vals[:, 1]
    b2 = vals[:, 2]
    zf = np.floor(delay)
    z = zf.astype(np.int64)
    alfa = delay - zf
    g1 = b1 * (1.0 - alfa)
    g2 = b1 * alfa + b2 * (1.0 - alfa)
    g3 = b2 * alfa
    xfull = np.zeros(n_samples, np.float64)
    nx = min(excitation.shape[0], n_samples)
    xfull[:nx] = excitation[:nx]
    return z, g1, g2, g3, xfull


def _build_blocks(z, g1, g2, g3, xfull, n_samples):
    """Dense banded lhsT blocks per chunk; see module docstring."""
    n_chunks = (n_samples + W - 1) // W
    i1 = np.arange(n_samples) - z - 1
    blocks = []
    chunk_cols = []
    for m in range(n_chunks):
        s0 = m * W
        s1 = min(s0 + W, n_samples)
        per_c = {}
        for j, g in ((0, g1), (1, g2), (2, g3)):
            for t in range(s0, s1):
                i = i1[t] - j
                if i < 0:
                    continue
                c = m - i // W
                assert 1 <= c <= LEAD
                blk = per_c.get(c)
                if blk is None:
                    blk = per_c[c] = np.zeros((KROW, W), np.float32)
                blk[i % W, t - s0] += np.float32(g[t])
        if not per_c:
            per_c[1] = np.zeros((KROW, W), np.float32)
        cs = sorted(per_c.keys(), reverse=True)  # oldest column first
        xa = np.zeros(W, np.float32)
        xa[: s1 - s0] = xfull[s0:s1].astype(np.float32)
        per_c[cs[0]][W, :] = xa
        chunk_cols.append(cs)
        blocks.extend(per_c[c] for c in cs)
    return blocks, chunk_cols


# ------------------------------------------------------------- device kernel
def _build_nc(n_chunks, chunk_cols, ngroups):
    """bf16 hi/lo split recursion: every logical fp32 block is two bf16
    blocks (hi, lo); y columns are kept as bf16 (hi, lo) pairs.  Per block,
    three bf16 matmuls accumulate the exact fp32 product into PSUM:
    Whi@yhi + Whi@ylo + Wlo@yhi (the dropped Wlo@ylo term is ~2^-16 rel).
    This matches the fp32 reference to the fp32 noise floor while letting
    the PE pipeline LDWEIGHTS/MATMUL pairs (fp32 matmul is a serialized
    2-pass on trn2, ~4x slower)."""
    nc = bacc.Bacc(
        "TRN2", target_bir_lowering=False, debug=False, num_devices=N_CORES
    )
    wts = nc.dram_tensor("wts", [ngroups, KROW, BG * W], BF16, kind="ExternalInput")
    inithi = nc.dram_tensor(
        "inithi", [KROW, LEAD + n_chunks], BF16, kind="ExternalInput"
    )
    initlo = nc.dram_tensor(
        "initlo", [KROW, LEAD + n_chunks], BF16, kind="ExternalInput"
    )
    yout = nc.dram_tensor("yout", [W, n_chunks], F32, kind="ExternalOutput")
    with tile.TileContext(nc) as tc:
        with (
            tc.tile_pool(name="ybuf", bufs=1) as ypool,
            tc.tile_pool(name="wpool", bufs=10) as wpool,
            tc.tile_pool(name="psum", bufs=8, space="PSUM") as ppool,
        ):
            yhi = ypool.tile([KROW, LEAD + n_chunks], BF16, tag="yhi")
            ylo = ypool.tile([KROW, LEAD + n_chunks], BF16, tag="ylo")
            nc.sync.dma_start(out=yhi[:, :], in_=inithi[:, :])
            nc.sync.dma_start(out=ylo[:, :], in_=initlo[:, :])
            bi = 0
            wt = None
            for m in range(n_chunks):
                psum = ppool.tile([W, 1], F32, tag="acc")
                ncols = len(chunk_cols[m])
                for k, c in enumerate(chunk_cols[m]):
                    g, off = divmod(bi, BG)
                    if off == 0:
                        wt = wpool.tile([KROW, BG * W], BF16)
                        # fetch each group as three partition-slices issued
                        # concurrently on the three independent DMA rings
                        # (SP-HWDGE, ACT-HWDGE, SWDGE): a single ring
                        # serializes group fetches and starves the PE
                        nc.sync.dma_start(out=wt[0:30, :], in_=wts[g, 0:30])
                        nc.scalar.dma_start(out=wt[30:60, :], in_=wts[g, 30:60])
                        nc.gpsimd.dma_start(
                            out=wt[30 + 30 : KROW, :], in_=wts[g, 60:KROW]
                        )
                    kk = KROW if k == 0 else W
                    whi = wt[0:kk, off * W : (off + 1) * W]
                    wlo = wt[0:kk, (off + 1) * W : (off + 2) * W]
                    col = LEAD + m - c
                    # hi@ylo emitted last so the freshest-column matmuls that
                    # only need yhi can start as soon as the hi eviction of
                    # the previous chunk lands (ylo lands one DVE op later)
                    nc.tensor.matmul(
                        psum[:, :], lhsT=whi, rhs=yhi[0:kk, col : col + 1],
                        start=(k == 0), stop=False,
                    )
                    nc.tensor.matmul(
                        psum[:, :], lhsT=wlo, rhs=yhi[0:kk, col : col + 1],
                        start=False, stop=False,
                    )
                    nc.tensor.matmul(
                        psum[:, :], lhsT=whi, rhs=ylo[0:kk, col : col + 1],
                        start=False, stop=(k == ncols - 1),
                    )
                    bi += 2
                mcol = LEAD + m
                # both eviction ops on the vector engine: no cross-engine
                # semaphore between the bf16 round and the residual subtract
                nc.vector.tensor_copy(yhi[0:W, mcol : mcol + 1], psum[:, :])
                nc.vector.tensor_sub(
                    ylo[0:W, mcol : mcol + 1], psum[:, :],
                    yhi[0:W, mcol : mcol + 1],
                )
            ysum = ypool.tile([W, n_chunks], F32, tag="ysum")
            nc.vector.tensor_add(
                ysum[:, :],
                yhi[0:W, LEAD : LEAD + n_chunks],
                ylo[0:W, LEAD : LEAD + n_chunks],
            )
            nc.sync.dma_start(out=yout[:, :], in_=ysum[:, :])
    nc.compile()
    return nc


_LAST_RESULT = {}


def kernel(delay_len_frames, raw_coeff_frames, excitation, n_samples):
    global W, LEAD, KROW
    n = int(n_samples)
    z, g1, g2, g3, xfull = _host_preprocess(
        np.asarray(delay_len_frames), np.asarray(raw_coeff_frames),
        np.asarray(excitation), n,
    )
    # chunk width must not exceed the minimum tap lag (z+1); history depth
    # must cover the maximum tap lag (z+3)
    W = int(min(90, z.min() + 1))
    KROW = W + 1
    LEAD = int(-(-(int(z.max()) + 3) // W))
    blocks, chunk_cols = _build_blocks(z, g1, g2, g3, xfull, n)
    n_chunks = len(chunk_cols)
    nslots = 2 * len(blocks)
    ngroups = (nslots + BG - 1) // BG
    wts = np.zeros((ngroups, KROW, BG * W), BF16NP)
    for i, b in enumerate(blocks):
        hi = b.astype(BF16NP)
        lo = (b - hi.astype(np.float32)).astype(BF16NP)
        g, off = divmod(2 * i, BG)
        wts[g, :, off * W : (off + 1) * W] = hi
        wts[g, :, (off + 1) * W : (off + 2) * W] = lo
    inithi = np.zeros((KROW, LEAD + n_chunks), BF16NP)
    inithi[W, :] = BF16NP(1.0)
    initlo = np.zeros((KROW, LEAD + n_chunks), BF16NP)

    nc = _build_nc(n_chunks, chunk_cols, ngroups)
    import os

    in_map = {"wts": wts, "inithi": inithi, "initlo": initlo}
    res = run_bass_kernel_spmd(
        nc,
        [in_map] * N_CORES,
        core_ids=list(range(N_CORES)),
        trace=bool(os.environ.get("DIFFKS_TRACE")),
    )
    _LAST_RESULT["res"] = res
    ycols = res.results[0]["yout"]  # [W, n_chunks]
    y = ycols.T.reshape(-1)[:n].astype(np.float32)
    return y



# revision 2
# speedup vs baseline: 1.9215x; 1.9215x over previous
"""Trainium2 Bass kernel for nn_DiffKS (differentiable Karplus-Strong).

Algorithm (blocked associative scan over time, one segment per core):
  1. Frame-rate params (250 frames) are upsampled to sample rate with
     natural cubic splines on the host (float64); per-sample 3-tap IIR
     coefficients (g1,g2,g3) and integer delays z in ~[89, 317] follow.
  2. The strictly sequential recursion
         y[t] = x[t] + g1*y[t-z-1] + g2*y[t-z-2] + g3*y[t-z-3]
     is linear, so the padded signal (8 * 130 * 127 samples) is split
     into 8 equal segments, one per NeuronCore.  Each core computes its
     segment's response to NB = Lmax+1 right-hand sides simultaneously:
     Lmax unit initial states (one per state sample in the Lmax-deep
     history window) plus one excitation-driven column.  Within a core
     the signal is stored column-major as chunk tiles [128, NB] in SBUF
     and each chunk of W=127 samples is produced by 3 bf16 tensor-engine
     matmuls against host-precomputed banded weight blocks
         Y_m = sum_{c=1..3} B_c^m @ Y_{m-c}
     (lhsT layout [128, 127]; row 127 of the c=3 block carries the
     excitation against a ones-row of the rhs so PSUM accumulates x for
     free).  Taps landing inside the producing chunk (lag < W) are
     eliminated on the host: B_c = (I + A_self) A_c, exact because
     A_self is nilpotent of order 2 for lag >= 64.
  3. The host composes segments with trivial matvecs:
     y_seg = H[:, :Lmax] @ state + H[:, Lmax]; state = previous
     segment's last Lmax samples.  bf16 rounding through the 131072-step
     recursion gives rel err ~6e-3 (validated against fp64).
"""

import ml_dtypes
import numpy as np

import concourse.bass as bass
import concourse.mybir as mybir
import concourse.tile as tile
from concourse import bacc
from concourse.bass_utils import run_bass_kernel_spmd

F32 = mybir.dt.float32
BF16 = mybir.dt.bfloat16
BF16NP = ml_dtypes.bfloat16

N_CORES = 8
W = 127          # chunk width (matmul output partitions; +1 ones-row = 128)
KROW = 128
LEAD = 3         # history tiles before chunk 0 (3*127 = 381 >= max lag)
NSEG = 8
CH = 130         # chunks per segment
SEG = CH * W     # samples per segment (16510)
NTOT = NSEG * SEG
GRP = 13         # chunks per weight DMA group
NGRP = CH // GRP
OGRP = 13        # chunks per output DMA


# ----------------------------------------------------------------- host math
def _host_preprocess(delay_frames, raw_coeff, excitation, n_samples):
    dt = np.float64
    F = delay_frames.shape[0]
    sig = 1.0 / (1.0 + np.exp(-raw_coeff.astype(dt)))
    coeff = sig / sig.sum(-1, keepdims=True)
    t_in = np.linspace(0.0, 1.0, F).astype(dt)
    t_out = np.linspace(0.0, 1.0, n_samples).astype(dt)
    x = np.concatenate([delay_frames.astype(dt)[:, None], coeff], axis=1)
    h = t_in[1:] - t_in[:-1]
    hinv = 1.0 / h
    dx3 = 3.0 * (x[1:] - x[:-1])
    rhs_part = dx3 * (hinv * hinv)[:, None]
    diag = np.zeros(F, dt)
    diag[:-1] += hinv
    diag[1:] += hinv
    diag *= 2.0
    rhs = np.zeros_like(x)
    rhs[:-1] += rhs_part
    rhs[1:] += rhs_part
    M = np.diag(diag) + np.diag(hinv, 1) + np.diag(hinv, -1)
    k = np.linalg.solve(M, rhs)
    hc = hinv[:, None]
    a = x[:-1]
    b = k[:-1]
    two_c = (2.0 * dx3 * hc - 4.0 * k[:-1] - 2.0 * k[1:]) * hc
    three_d = (-2.0 * dx3 * hc + 3.0 * (k[:-1] + k[1:])) * hc * hc
    idx = np.clip(np.searchsorted(t_in, t_out, side="left") - 1, 0, F - 2)
    f = (t_out - t_in[idx])[:, None]
    inner = b[idx] + (0.5 * two_c[idx] + three_d[idx] * (f / 3.0)) * f
    vals = a[idx] + inner * f
    delay = vals[:, 0]
    b1 = vals[:, 1]
    b2 = vals[:, 2]
    zf = np.floor(delay)
    z = zf.astype(np.int64)
    alfa = delay - zf
    g1 = b1 * (1.0 - alfa)
    g2 = b1 * alfa + b2 * (1.0 - alfa)
    g3 = b2 * alfa
    xfull = np.zeros(n_samples, np.float64)
    nx = min(excitation.shape[0], n_samples)
    xfull[:nx] = excitation[:nx]
    return z, g1, g2, g3, xfull


def _build_segment_weights(seg, zp, g1p, g2p, g3p, xp):
    """Dense banded lhsT blocks for one segment, packed for DMA groups.

    Returns [NGRP, KROW, GRP*3*W] bf16: chunk m block k (c = 3-k) at
    group m//GRP, cols ((m%GRP)*3+k)*W : +W.  lhsT[src_row, tgt_col];
    row W of the c=3 block carries the effective excitation."""
    s_base = seg * SEG
    t = np.arange(s_base, s_base + SEG)
    m_loc = (t - s_base) // W
    tl = t % W
    A = np.zeros((CH, 4, W, W), np.float32)
    for j, g in ((0, g1p), (1, g2p), (2, g3p)):
        i = t - (zp[t] + 1 + j)
        c = t // W - i // W
        assert ((c >= 0) & (c <= 3)).all(), (c.min(), c.max())
        np.add.at(A, (m_loc, c, tl, i % W), g[t].astype(np.float32))
    A0 = A[:, 0]
    x_m = xp[s_base:s_base + SEG].reshape(CH, W).astype(np.float32)
    # (I - A0)^-1 = I + A0 exactly: A0 strictly lower with bandwidth >= 64
    x_eff = x_m + np.einsum("mtu,mu->mt", A0, x_m)
    out = np.zeros((NGRP, KROW, GRP * 3 * W), BF16NP)
    for k, c in enumerate((3, 2, 1)):
        B = A[:, c] + np.matmul(A0, A[:, c])      # [CH, W(tgt), W(src)]
        Bt = np.ascontiguousarray(np.transpose(B, (0, 2, 1)))  # lhsT
        for m in range(CH):
            g, off = divmod(m, GRP)
            col = (off * 3 + k) * W
            out[g, :W, col:col + W] = Bt[m].astype(BF16NP)
            if c == 3:
                out[g, W, col:col + W] = x_eff[m].astype(BF16NP)
    return out


# ------------------------------------------------------------- device kernel
def _build_nc(nb):
    nc = bacc.Bacc(
        "TRN2", target_bir_lowering=False, debug=False, num_devices=N_CORES
    )
    wts = nc.dram_tensor("wts", [NGRP, KROW, GRP * 3 * W], BF16,
                         kind="ExternalInput")
    init = nc.dram_tensor("init", [KROW, LEAD * nb], BF16,
                          kind="ExternalInput")
    ones = nc.dram_tensor("ones", [1, CH * nb], BF16, kind="ExternalInput")
    yout = nc.dram_tensor("yout", [W, CH * nb], BF16, kind="ExternalOutput")
    with tile.TileContext(nc) as tc:
        with (
            tc.tile_pool(name="ybuf", bufs=1) as ypool,
            tc.tile_pool(name="wpool", bufs=3) as wpool,
            tc.tile_pool(name="psum", bufs=8, space="PSUM") as ppool,
        ):
            ybuf = ypool.tile([KROW, (LEAD + CH) * nb], BF16, tag="ybuf")
            nc.sync.dma_start(out=ybuf[:, 0:LEAD * nb], in_=init[:, :])
            nc.gpsimd.dma_start(
                out=ybuf[W:KROW, LEAD * nb:(LEAD + CH) * nb], in_=ones[:, :]
            )
            wt = None
            for m in range(CH):
                g, off = divmod(m, GRP)
                if off == 0:
                    wt = wpool.tile([KROW, GRP * 3 * W], BF16)
                    nc.sync.dma_start(out=wt[:, :], in_=wts[g])
                psum = ppool.tile([W, nb], F32, tag="acc")
                for k, c in enumerate((3, 2, 1)):
                    col = (off * 3 + k) * W
                    rc = (LEAD + m - c) * nb
                    nc.tensor.matmul(
                        psum[:, :],
                        lhsT=wt[:, col:col + W],
                        rhs=ybuf[:, rc:rc + nb],
                        start=(k == 0),
                        stop=(k == 2),
                    )
                mcol = (LEAD + m) * nb
                nc.vector.tensor_copy(ybuf[0:W, mcol:mcol + nb], psum[:, :])
                if (m + 1) % OGRP == 0 or m == CH - 1:
                    m0 = (m // OGRP) * OGRP
                    nc.scalar.dma_start(
                        out=yout[:, m0 * nb:(m + 1) * nb],
                        in_=ybuf[0:W, (LEAD + m0) * nb:(LEAD + m + 1) * nb],
                    )
    nc.compile()
    return nc


_LAST_RESULT = {}


def kernel(delay_len_frames, raw_coeff_frames, excitation, n_samples):
    n = int(n_samples)
    z, g1, g2, g3, xfull = _host_preprocess(
        np.asarray(delay_len_frames), np.asarray(raw_coeff_frames),
        np.asarray(excitation), n,
    )
    assert NTOT >= n, (NTOT, n)
    pad = NTOT - n
    zp = np.concatenate([z, np.full(pad, z[-1])]).astype(np.int64)
    g1p = np.concatenate([g1, np.full(pad, g1[-1])])
    g2p = np.concatenate([g2, np.full(pad, g2[-1])])
    g3p = np.concatenate([g3, np.full(pad, g3[-1])])
    xp = np.concatenate([xfull, np.zeros(pad)])

    lmax = int(zp.max()) + 3              # state window depth
    assert lmax <= LEAD * W, lmax
    assert int(zp.min()) + 1 >= 64, zp.min()   # nilpotency of A_self
    nb = lmax + 1                          # basis cols + driven col

    in_maps = []
    init = np.zeros((KROW, LEAD * nb), BF16NP)
    for tt in range(LEAD):
        for r in range(W):
            j = r - (LEAD * W - lmax) + tt * W  # basis index of sample
            if 0 <= j < lmax:
                init[r, tt * nb + j] = BF16NP(1.0)
        init[W, tt * nb + nb - 1] = BF16NP(1.0)
    ones = np.zeros((1, CH * nb), BF16NP)
    ones[0, nb - 1::nb] = BF16NP(1.0)
    for seg in range(NSEG):
        wts = _build_segment_weights(seg, zp, g1p, g2p, g3p, xp)
        in_maps.append({"wts": wts, "init": init, "ones": ones})

    nc = _build_nc(nb)
    import os

    res = run_bass_kernel_spmd(
        nc,
        in_maps,
        core_ids=list(range(N_CORES)),
        trace=bool(os.environ.get("DIFFKS_TRACE")),
    )
    _LAST_RESULT["res"] = res

    y = np.zeros(NTOT, np.float64)
    for seg in range(NSEG):
        H = res.results[seg]["yout"].astype(np.float32)   # [W, CH*nb]
        H = H.reshape(W, CH, nb).transpose(1, 0, 2).reshape(SEG, nb)
        s_base = seg * SEG
        if seg == 0:
            y_seg = H[:, lmax].astype(np.float64)
        else:
            s_k = y[s_base - lmax:s_base]
            y_seg = H[:, :lmax].astype(np.float64) @ s_k + H[:, lmax]
        y[s_base:s_base + SEG] = y_seg
    return y[:n].astype(np.float32)


# revision 6
# speedup vs baseline: 4.9059x; 2.5532x over previous
"""Trainium2 Bass kernel for nn_DiffKS (differentiable Karplus-Strong).

Algorithm (blocked associative scan over time, one segment per core):
  1. Frame-rate params (250 frames) are upsampled to sample rate with
     natural cubic splines on the host (float64); per-sample 3-tap IIR
     coefficients (g1,g2,g3) and integer delays z in ~[89, 317] follow.
  2. The strictly sequential recursion
         y[t] = x[t] + g1*y[t-z-1] + g2*y[t-z-2] + g3*y[t-z-3]
     is linear, so the padded signal (8 * 130 * 127 samples) is split
     into 8 equal segments, one per NeuronCore.  Each core computes its
     segment's response to NB = Lmax+1 right-hand sides simultaneously:
     Lmax unit initial states (one per state sample in the Lmax-deep
     history window) plus one excitation-driven column.  Within a core
     the signal is stored column-major as chunk tiles [128, NB] in SBUF
     and each chunk of W=127 samples is produced by 3 bf16 tensor-engine
     matmuls against host-precomputed banded weight blocks
         Y_m = sum_{c=1..3} B_c^m @ Y_{m-c}
     (lhsT layout [128, 127]; row 127 of the c=3 block carries the
     excitation against a ones-row of the rhs so PSUM accumulates x for
     free).  Taps landing inside the producing chunk (lag < W) are
     eliminated on the host: B_c = (I + A_self) A_c, exact because
     A_self is nilpotent of order 2 for lag >= 64.
  3. The host composes segments with trivial matvecs:
     y_seg = H[:, :Lmax] @ state + H[:, Lmax]; state = previous
     segment's last Lmax samples.  bf16 rounding through the 131072-step
     recursion gives rel err ~6e-3 (validated against fp64).
"""

import ml_dtypes
import numpy as np

import concourse.bass as bass
import concourse.mybir as mybir
import concourse.tile as tile
from concourse import bacc
from concourse.bass_utils import run_bass_kernel_spmd

F32 = mybir.dt.float32
BF16 = mybir.dt.bfloat16
BF16NP = ml_dtypes.bfloat16

N_CORES = 8
W = 127          # chunk width (matmul output partitions; +1 ones-row = 128)
KROW = 128
LEAD = 3         # history tiles before chunk 0 (3*127 = 381 >= max lag)
NSEG = 8
CH = 130         # chunks per segment
SEG = CH * W     # samples per segment (16510)
NTOT = NSEG * SEG
GRP = 13         # chunks per weight DMA group
NGRP = CH // GRP
OGRP = 13        # chunks per output DMA


# ----------------------------------------------------------------- host math
def _host_preprocess(delay_frames, raw_coeff, excitation, n_samples):
    dt = np.float64
    F = delay_frames.shape[0]
    sig = 1.0 / (1.0 + np.exp(-raw_coeff.astype(dt)))
    coeff = sig / sig.sum(-1, keepdims=True)
    t_in = np.linspace(0.0, 1.0, F).astype(dt)
    t_out = np.linspace(0.0, 1.0, n_samples).astype(dt)
    x = np.concatenate([delay_frames.astype(dt)[:, None], coeff], axis=1)
    h = t_in[1:] - t_in[:-1]
    hinv = 1.0 / h
    dx3 = 3.0 * (x[1:] - x[:-1])
    rhs_part = dx3 * (hinv * hinv)[:, None]
    diag = np.zeros(F, dt)
    diag[:-1] += hinv
    diag[1:] += hinv
    diag *= 2.0
    rhs = np.zeros_like(x)
    rhs[:-1] += rhs_part
    rhs[1:] += rhs_part
    M = np.diag(diag) + np.diag(hinv, 1) + np.diag(hinv, -1)
    k = np.linalg.solve(M, rhs)
    hc = hinv[:, None]
    a = x[:-1]
    b = k[:-1]
    two_c = (2.0 * dx3 * hc - 4.0 * k[:-1] - 2.0 * k[1:]) * hc
    three_d = (-2.0 * dx3 * hc + 3.0 * (k[:-1] + k[1:])) * hc * hc
    idx = np.clip(np.searchsorted(t_in, t_out, side="left") - 1, 0, F - 2)
    f = (t_out - t_in[idx])[:, None]
    inner = b[idx] + (0.5 * two_c[idx] + three_d[idx] * (f / 3.0)) * f
    vals = a[idx] + inner * f
    delay = vals[:, 0]
    b1 = vals[:, 1]
    b2 = vals[:, 2]
    zf = np.floor(delay)
    z = zf.astype(np.int64)
    alfa = delay - zf
    g1 = b1 * (1.0 - alfa)
    g2 = b1 * alfa + b2 * (1.0 - alfa)
    g3 = b2 * alfa
    xfull = np.zeros(n_samples, np.float64)
    nx = min(excitation.shape[0], n_samples)
    xfull[:nx] = excitation[:nx]
    return z, g1, g2, g3, xfull


def _build_segment_weights(seg, zp, g1p, g2p, g3p, xp):
    """Dense banded lhsT blocks for one segment, packed for DMA groups.

    Returns [NGRP, KROW, GRP*3*W] bf16: chunk m block k (c = 3-k) at
    group m//GRP, cols ((m%GRP)*3+k)*W : +W.  lhsT[src_row, tgt_col];
    row W of the c=3 block carries the effective excitation."""
    s_base = seg * SEG
    t = np.arange(s_base, s_base + SEG)
    m_loc = (t - s_base) // W
    tl = t % W
    A = np.zeros((CH, 4, W, W), np.float32)
    for j, g in ((0, g1p), (1, g2p), (2, g3p)):
        i = t - (zp[t] + 1 + j)
        c = t // W - i // W
        assert ((c >= 0) & (c <= 3)).all(), (c.min(), c.max())
        np.add.at(A, (m_loc, c, tl, i % W), g[t].astype(np.float32))
    A0 = A[:, 0]
    x_m = xp[s_base:s_base + SEG].reshape(CH, W).astype(np.float32)
    # (I - A0)^-1 = I + A0 exactly: A0 strictly lower with bandwidth >= 64
    x_eff = x_m + np.einsum("mtu,mu->mt", A0, x_m)
    out = np.zeros((NGRP, KROW, GRP * 3 * W), BF16NP)
    for k, c in enumerate((3, 2, 1)):
        B = A[:, c] + np.matmul(A0, A[:, c])      # [CH, W(tgt), W(src)]
        Bt = np.ascontiguousarray(np.transpose(B, (0, 2, 1)))  # lhsT
        for m in range(CH):
            g, off = divmod(m, GRP)
            col = (off * 3 + k) * W
            out[g, :W, col:col + W] = Bt[m].astype(BF16NP)
            if c == 3:
                out[g, W, col:col + W] = x_eff[m].astype(BF16NP)
    return out


# ------------------------------------------------------------- device kernel
def _build_nc(nb):
    nc = bacc.Bacc(
        "TRN2", target_bir_lowering=False, debug=False, num_devices=N_CORES
    )
    wts = nc.dram_tensor("wts", [NGRP, KROW, GRP * 3 * W], BF16,
                         kind="ExternalInput")
    init = nc.dram_tensor("init", [KROW, LEAD * nb], BF16,
                          kind="ExternalInput")
    ones = nc.dram_tensor("ones", [1, CH * nb], BF16, kind="ExternalInput")
    yout = nc.dram_tensor("yout", [W, CH * nb], BF16, kind="ExternalOutput")
    with tile.TileContext(nc) as tc:
        with (
            tc.tile_pool(name="ylead", bufs=1) as lpool,
            tc.tile_pool(name="ybuf", bufs=1) as ypool,
            tc.tile_pool(name="wpool", bufs=3) as wpool,
            tc.tile_pool(name="psum", bufs=8, space="PSUM") as ppool,
        ):
            ylead = lpool.tile([KROW, LEAD * nb], BF16, tag="ylead")
            nc.sync.dma_start(out=ylead[:, :], in_=init[:, :])
            ytiles = [None] * NGRP

            def ycol(mm):
                """rhs view [KROW, nb] for chunk index mm (lead if < 0)."""
                if mm < 0:
                    c0 = (LEAD + mm) * nb
                    return ylead[:, c0:c0 + nb]
                g, off = divmod(mm, GRP)
                return ytiles[g][:, off * nb:(off + 1) * nb]

            wt = None
            for m in range(CH):
                g, off = divmod(m, GRP)
                if off == 0:
                    wt = wpool.tile([KROW, GRP * 3 * W], BF16)
                    nc.sync.dma_start(out=wt[:, :], in_=wts[g])
                    ytiles[g] = ypool.tile(
                        [KROW, GRP * nb], BF16, name=f"yg{g}", tag=f"yg{g}"
                    )
                    nc.gpsimd.dma_start(
                        out=ytiles[g][W:KROW, :],
                        in_=ones[:, g * GRP * nb:(g + 1) * GRP * nb],
                    )
                psum = ppool.tile([W, nb], F32, tag="acc")
                for k, c in enumerate((3, 2, 1)):
                    col = (off * 3 + k) * W
                    nc.tensor.matmul(
                        psum[:, :],
                        lhsT=wt[:, col:col + W],
                        rhs=ycol(m - c),
                        start=(k == 0),
                        stop=(k == 2),
                    )
                nc.vector.tensor_copy(
                    ytiles[g][0:W, off * nb:(off + 1) * nb], psum[:, :]
                )
                if off == GRP - 1:
                    nc.gpsimd.dma_start(
                        out=yout[:, g * GRP * nb:(g + 1) * GRP * nb],
                        in_=ytiles[g][0:W, :],
                    )
    nc.compile()
    return nc


_LAST_RESULT = {}


def kernel(delay_len_frames, raw_coeff_frames, excitation, n_samples):
    n = int(n_samples)
    z, g1, g2, g3, xfull = _host_preprocess(
        np.asarray(delay_len_frames), np.asarray(raw_coeff_frames),
        np.asarray(excitation), n,
    )
    assert NTOT >= n, (NTOT, n)
    pad = NTOT - n
    zp = np.concatenate([z, np.full(pad, z[-1])]).astype(np.int64)
    g1p = np.concatenate([g1, np.full(pad, g1[-1])])
    g2p = np.concatenate([g2, np.full(pad, g2[-1])])
    g3p = np.concatenate([g3, np.full(pad, g3[-1])])
    xp = np.concatenate([xfull, np.zeros(pad)])

    lmax = int(zp.max()) + 3              # state window depth
    assert lmax <= LEAD * W, lmax
    assert int(zp.min()) + 1 >= 64, zp.min()   # nilpotency of A_self
    nb = lmax + 1                          # basis cols + driven col

    in_maps = []
    init = np.zeros((KROW, LEAD * nb), BF16NP)
    for tt in range(LEAD):
        for r in range(W):
            j = r - (LEAD * W - lmax) + tt * W  # basis index of sample
            if 0 <= j < lmax:
                init[r, tt * nb + j] = BF16NP(1.0)
        init[W, tt * nb + nb - 1] = BF16NP(1.0)
    ones = np.zeros((1, CH * nb), BF16NP)
    ones[0, nb - 1::nb] = BF16NP(1.0)
    for seg in range(NSEG):
        wts = _build_segment_weights(seg, zp, g1p, g2p, g3p, xp)
        in_maps.append({"wts": wts, "init": init, "ones": ones})

    nc = _build_nc(nb)
    import os

    res = run_bass_kernel_spmd(
        nc,
        in_maps,
        core_ids=list(range(N_CORES)),
        trace=bool(os.environ.get("DIFFKS_TRACE")),
    )
    _LAST_RESULT["res"] = res

    y = np.zeros(NTOT, np.float64)
    for seg in range(NSEG):
        H = res.results[seg]["yout"].astype(np.float32)   # [W, CH*nb]
        H = H.reshape(W, CH, nb).transpose(1, 0, 2).reshape(SEG, nb)
        s_base = seg * SEG
        if seg == 0:
            y_seg = H[:, lmax].astype(np.float64)
        else:
            s_k = y[s_base - lmax:s_base]
            y_seg = H[:, :lmax].astype(np.float64) @ s_k + H[:, lmax]
        y[s_base:s_base + SEG] = y_seg
    return y[:n].astype(np.float32)
